# revision 1
# baseline (speedup 1.0000x reference)
"""Grouped-experts SwiGLU MLP on 8 TRN2 NeuronCores, expert-parallel.

Per core (one expert):
    g = x @ gate; u = x @ down; h = silu(g) * u; out = h @ up
with T=2048, D_IN=2048, D_OUT=4096 (three 2048x2048x4096 matmuls).

Layout strategy: the host pre-transposes x[e] -> xT [D, T] so every matmul
operand arrives in its natural layout (contraction dim on partitions):
  MM1/2: psum[h,t] += gate[d,h].T-block @ xT[d,t]   (lhsT=gate tile, rhs=xT)
  MM3:   psum[t,d] += hT[h,t]-block    @ up[h,d]    (lhsT=hT tile,  rhs=up)
hT (silu(g)*u, [H, T]) is staged in DRAM between the two phases (too big
for SBUF at fp32 alongside resident xT).

All matmuls run in float32r (reduced-precision fp32, 1 PE cycle/row at
moving-dim 512 == bf16 rate; measured rel-err ~2e-4 vs fp32).
"""
import sys

if "/opt/trn_rl_repo" not in sys.path:
    sys.path.insert(0, "/opt/trn_rl_repo")

import numpy as np

import bass_rust
import concourse.bass as bass
import concourse.mybir as mybir
import concourse.tile as tile
from concourse.bass_utils import run_bass_kernel_spmd

E, T, D, H = 8, 2048, 2048, 4096
P = 128
KD = D // P   # 16 contraction chunks for MM1/2
KH = H // P   # 32 contraction chunks for MM3
NT = T // P   # 16 token stripes
F32 = mybir.dt.float32
F32R = mybir.dt.float32r
SILU = mybir.ActivationFunctionType.Silu
COPY = mybir.ActivationFunctionType.Copy


def _split_multi_waits(nc, max_waits=1):
    """This walrus build rejects instructions with >1 sync wait ("Too many
    sync wait commands").  Hoist extra waits onto single-wait NOPs on the
    same engine, placed immediately before the offending instruction."""
    ctr = 0
    for f in nc.m.functions:
        for blk in f.blocks:
            out = []
            changed = False
            for inst in blk.instructions:
                si = inst.sync_info
                waits = list(si.on_wait) if si is not None and si.on_wait else []
                if len(waits) > max_waits:
                    for w in waits[:-max_waits]:
                        ctr += 1
                        n = bass_rust.InstNoOp(name=f"I-wsplit-{ctr}")
                        n.engine = inst.engine
                        n.sync_info = bass_rust.SyncInfo(on_wait=[w], on_update=[])
                        out.append(n)
                    inst.sync_info = bass_rust.SyncInfo(
                        on_wait=waits[-max_waits:],
                        on_update=list(si.on_update or []),
                    )
                    changed = True
                out.append(inst)
            if changed:
                blk.instructions = out
    return ctr


def _build():
    nc = bass.Bass()
    xt_ext = nc.declare_dram_parameter("xt", [D, T], F32, isOutput=False)
    gate_ext = nc.declare_dram_parameter("gate", [D, H], F32, isOutput=False)
    down_ext = nc.declare_dram_parameter("down", [D, H], F32, isOutput=False)
    up_ext = nc.declare_dram_parameter("up", [H, D], F32, isOutput=False)
    out_ext = nc.declare_dram_parameter("out", [T, D], F32, isOutput=True)
    ht_dram = nc.dram_tensor("ht", [H, T], F32R)

    xt_r = xt_ext[:, :].rearrange("(k p) t -> p k t", p=P).bitcast(F32R)
    gate_r = gate_ext[:, :].rearrange("(k p) h -> p k h", p=P).bitcast(F32R)
    down_r = down_ext[:, :].rearrange("(k p) h -> p k h", p=P).bitcast(F32R)
    up_r = up_ext[:, :].rearrange("(k p) d -> p k d", p=P).bitcast(F32R)

    with tile.TileContext(nc) as tc:
        # ---- Phase 1: hT[h, t] = silu(g) * u, streamed over 32 h-stripes
        with tc.tile_pool(name="xpool", bufs=1) as xpool, \
             tc.tile_pool(name="wpool", bufs=2) as wpool, \
             tc.tile_pool(name="hpool", bufs=2) as hpool, \
             tc.tile_pool(name="spool", bufs=2) as spool, \
             tc.tile_pool(name="ps1", bufs=8, space="PSUM") as ps1:
            # Issue stripe-0 weight loads before the bulk xT load so the first
            # matmuls aren't queued behind 16 MB of xT DMA on the same lanes.
            xbuf = xpool.tile([P, KD, T], F32R)
            gbuf0 = wpool.tile([P, KD, P], F32R, tag="gbuf", name="gbuf0")
            dbuf0 = wpool.tile([P, KD, P], F32R, tag="dbuf", name="dbuf0")
            nc.sync.dma_start(out=gbuf0[:, :, :], in_=gate_r[:, :, 0:P])
            nc.sync.dma_start(out=dbuf0[:, :, :], in_=down_r[:, :, 0:P])
            for k in range(KD):
                nc.sync.dma_start(out=xbuf[:, k, :], in_=xt_r[:, k, :])

            for i in range(KH):
                if i == 0:
                    gbuf, dbuf = gbuf0, dbuf0
                else:
                    gbuf = wpool.tile([P, KD, P], F32R, tag="gbuf", name=f"gbuf{i}")
                    dbuf = wpool.tile([P, KD, P], F32R, tag="dbuf", name=f"dbuf{i}")
                    nc.sync.dma_start(out=gbuf[:, :, :], in_=gate_r[:, :, i * P:(i + 1) * P])
                    nc.sync.dma_start(out=dbuf[:, :, :], in_=down_r[:, :, i * P:(i + 1) * P])

                pg = [ps1.tile([P, 512], F32, tag="ps1", name=f"pg{t4}") for t4 in range(4)]
                pd = [ps1.tile([P, 512], F32, tag="ps1", name=f"pd{t4}") for t4 in range(4)]
                for k in range(KD):
                    st, sp = k == 0, k == KD - 1
                    for t4 in range(4):
                        nc.tensor.matmul(
                            pg[t4][:, :], lhsT=gbuf[:, k, :],
                            rhs=xbuf[:, k, t4 * 512:(t4 + 1) * 512],
                            start=st, stop=sp,
                        )
                    for t4 in range(4):
                        nc.tensor.matmul(
                            pd[t4][:, :], lhsT=dbuf[:, k, :],
                            rhs=xbuf[:, k, t4 * 512:(t4 + 1) * 512],
                            start=st, stop=sp,
                        )

                hbuf = hpool.tile([P, T], F32R)
                for t4 in range(4):
                    sg = spool.tile([P, 512], F32)
                    nc.scalar.activation(out=sg[:, :], in_=pg[t4][:, :], func=SILU, scale=1.0)
                    nc.vector.tensor_mul(
                        hbuf[:, t4 * 512:(t4 + 1) * 512], pd[t4][:, :], sg[:, :]
                    )
                nc.sync.dma_start(out=ht_dram[i * P:(i + 1) * P, :], in_=hbuf[:, :])

        # ---- Phase 2: out[t, d] = sum_h hT[h, t] * up[h, d]
        with tc.tile_pool(name="upool", bufs=1) as upool, \
             tc.tile_pool(name="cpool", bufs=3) as cpool, \
             tc.tile_pool(name="opool", bufs=4) as opool, \
             tc.tile_pool(name="ps2", bufs=8, space="PSUM") as ps2:
            for dh in range(2):
                # Load t=0's hT column before the 16 MB of up tiles so the
                # first matmuls aren't queued behind them; per-k up tiles let
                # the k-loop start as soon as the first 512 KB chunk lands.
                htcol0 = cpool.tile([P, KH, P], F32R, tag="htcol", name=f"htcol{dh}_0")
                ht_src0 = ht_dram[:, 0:P].rearrange("(k p) j -> p k j", p=P)
                for kc in range(4):
                    nc.sync.dma_start(
                        out=htcol0[:, kc * 8:(kc + 1) * 8, :],
                        in_=ht_src0[:, kc * 8:(kc + 1) * 8, :],
                    )
                upk = [
                    upool.tile([P, 1024], F32R, tag=f"upk{k}", name=f"up{dh}_{k}")
                    for k in range(KH)
                ]
                for k in range(KH):
                    nc.sync.dma_start(
                        out=upk[k][:, :],
                        in_=up_r[:, k, dh * 1024:(dh + 1) * 1024],
                    )
                for t in range(NT):
                    if t == 0:
                        htcol = htcol0
                    else:
                        htcol = cpool.tile([P, KH, P], F32R, tag="htcol", name=f"htcol{dh}_{t}")
                        ht_src = ht_dram[:, t * P:(t + 1) * P].rearrange(
                            "(k p) j -> p k j", p=P
                        )
                        for kc in range(4):
                            nc.sync.dma_start(
                                out=htcol[:, kc * 8:(kc + 1) * 8, :],
                                in_=ht_src[:, kc * 8:(kc + 1) * 8, :],
                            )
                    po = [ps2.tile([P, 512], F32, tag="ps2", name=f"po{d5}") for d5 in range(2)]
                    for k in range(KH):
                        st, sp = k == 0, k == KH - 1
                        for d5 in range(2):
                            nc.tensor.matmul(
                                po[d5][:, :], lhsT=htcol[:, k, :],
                                rhs=upk[k][:, d5 * 512:(d5 + 1) * 512],
                                start=st, stop=sp,
                            )
                    for d5 in range(2):
                        oc = opool.tile([P, 512], F32)
                        nc.scalar.activation(out=oc[:, :], in_=po[d5][:, :], func=COPY, scale=1.0)
                        nc.sync.dma_start(
                            out=out_ext[t * P:(t + 1) * P,
                                        dh * 1024 + d5 * 512:dh * 1024 + (d5 + 1) * 512],
                            in_=oc[:, :],
                        )

    _split_multi_waits(nc)
    return nc


_NC = None


def kernel(x, gate_proj, down_proj, up_proj, **run_kwargs):
    global _NC
    if _NC is None:
        _NC = _build()
    in_maps = []
    for e in range(E):
        in_maps.append({
            "xt": np.ascontiguousarray(np.asarray(x[e], dtype=np.float32).T),
            "gate": np.ascontiguousarray(np.asarray(gate_proj[e], dtype=np.float32)),
            "down": np.ascontiguousarray(np.asarray(down_proj[e], dtype=np.float32)),
            "up": np.ascontiguousarray(np.asarray(up_proj[e], dtype=np.float32)),
        })
    res = run_bass_kernel_spmd(_NC, in_maps, core_ids=list(range(E)), **run_kwargs)
    out = np.stack([res.results[e]["out"] for e in range(E)]).astype(np.float32)
    if run_kwargs:
        kernel.last_result = res
    return out



# revision 36
# speedup vs baseline: 1.1317x; 1.1317x over previous
"""Grouped-experts SwiGLU MLP on 8 TRN2 NeuronCores, expert-parallel, bf16.

Per core (one expert):
    g = x @ gate; u = x @ down; h = silu(g) * u; out = h @ up
with T=2048, D_IN=2048, D_OUT=4096 (three 2048x2048x4096 matmuls).

Layout strategy (1348us vs the 1534us fp32r baseline; PE-roofline-bound):
  - Everything bf16 (weights, x, hT, staged out). End-to-end error ~4.6e-3
    vs the 2e-2 gate. bf16 LDWEIGHTS (97ns) hides fully under the 512-row
    matmul stream -> 216ns/matmul cadence (fp32r's 187ns load was partially
    exposed, inflating the cadence to 227-249ns).
  - hT = silu(g)*u stays RESIDENT in SBUF ([128, 32, 2048] bf16 =
    128KB/partition) -- no 64MB DRAM round-trip, no phase-boundary DMA.
  - Phase 1 runs in two T-halves with xT half-resident (32KB/p); gate/down
    are streamed twice in a host-packed [p, stripe, k, col] layout so each
    stripe load is one DMA of 4KB-contiguous per-partition rows.
  - Phase 2 computes outT = up.T-blocks @ hT with d-quarters of up
    streamed (double-buffered), 4 matmuls per weight load; out written
    transposed bf16 and flipped/upcast on host.
  - A 14-matmul warmup covers the HAM clock-gate window + DMA lead-in; one
    PSUM pool spans both phases (no drain-WAR at the barrier); the first
    12 up-chunks preload into a phase-spanning pool.
  - Rejected: fp8 DoubleRow is 2x FLOPs at best on HW (not the cost
    model's 4x); the 3-term error-compensated split needed to pass 2e-2
    would be 1.5x slower than bf16.
"""
import sys

if "/opt/trn_rl_repo" not in sys.path:
    sys.path.insert(0, "/opt/trn_rl_repo")

import numpy as np
import ml_dtypes

import bass_rust
import concourse.bass as bass
import concourse.mybir as mybir
import concourse.tile as tile
from concourse.bass_utils import run_bass_kernel_spmd

E, T, D, H = 8, 2048, 2048, 4096
P = 128
KD = D // P   # 16 contraction chunks for MM1/2
KH = H // P   # 32 contraction chunks for MM3
F32 = mybir.dt.float32
BF16 = mybir.dt.bfloat16
SILU = mybir.ActivationFunctionType.Silu
COPY = mybir.ActivationFunctionType.Copy
NPBF = ml_dtypes.bfloat16


def _split_multi_waits(nc, max_waits=1):
    """This walrus build rejects instructions with >1 sync wait ("Too many
    sync wait commands").  Hoist extra waits onto single-wait NOPs on the
    same engine, placed immediately before the offending instruction."""
    ctr = 0
    for f in nc.m.functions:
        for blk in f.blocks:
            out = []
            changed = False
            for inst in blk.instructions:
                si = inst.sync_info
                waits = list(si.on_wait) if si is not None and si.on_wait else []
                if len(waits) > max_waits:
                    for w in waits[:-max_waits]:
                        ctr += 1
                        n = bass_rust.InstNoOp(name=f"I-wsplit-{ctr}")
                        n.engine = inst.engine
                        n.sync_info = bass_rust.SyncInfo(on_wait=[w], on_update=[])
                        out.append(n)
                    inst.sync_info = bass_rust.SyncInfo(
                        on_wait=waits[-max_waits:],
                        on_update=list(si.on_update or []),
                    )
                    changed = True
                out.append(inst)
            if changed:
                blk.instructions = out
    return ctr


def _build(split_waits=True):
    nc = bass.Bass()
    xt_ext = nc.declare_dram_parameter("xt", [D, T], BF16, isOutput=False)
    g3_ext = nc.declare_dram_parameter("g3", [P, KH * KD * P], BF16, isOutput=False)
    d3_ext = nc.declare_dram_parameter("d3", [P, KH * KD * P], BF16, isOutput=False)
    up_ext = nc.declare_dram_parameter("up", [H, D], BF16, isOutput=False)
    outT_ext = nc.declare_dram_parameter("outT", [D, T], BF16, isOutput=True)

    xt_r = xt_ext[:, :].rearrange("(k p) t -> p k t", p=P)
    up_r = up_ext[:, :].rearrange("(k p) d -> p k d", p=P)

    with tile.TileContext(nc) as tc:
        # One PSUM pool spans both phases: phase 2's first po tiles rotate
        # onto banks drained two stripes before the phase boundary, instead
        # of WARing against the final stripe's silu/mul drain.
        with tc.tile_pool(name="htpool", bufs=1) as htpool, \
             tc.tile_pool(name="uhead", bufs=1) as uhpool, \
             tc.tile_pool(name="psp", bufs=8, space="PSUM") as psp:
            htbuf = htpool.tile([P, KH, T], BF16, name="htbuf")
            # First 12 up k-chunks for phase 2's first d-quarter, loaded during
            # phase 1 from a pool that outlives the phase-1 pools -- phase 2's
            # first matmuls don't wait on the pool-ring space release.
            UH = 12
            uhead = uhpool.tile([P, UH, 512], BF16, name="uhead")

            # ---- Phase 1: htbuf[h, t] = silu(x@gate) * (x@down), bf16
            with tc.tile_pool(name="xpool", bufs=1) as xpool, \
                 tc.tile_pool(name="wpool", bufs=2) as wpool, \
                 tc.tile_pool(name="spool", bufs=2) as spool:
                ps1 = psp
                # PE warmup during the DMA lead-in: 14 cold matmuls (~427ns
                # each) cover the 3.4us HAM activity window AND span the
                # ~6us until the first weight/x transfers land, so real
                # matmuls start immediately at the full 2.4GHz clock. The
                # result is never consumed.
                wz = spool.tile([P, 512], BF16, tag="wz", name="wz")
                nc.vector.memset(wz[:, :], 0.0)
                pwarm = ps1.tile([P, 512], F32, tag="ps", name="pwarm")
                for w in range(14):
                    nc.tensor.matmul(
                        pwarm[:, :], lhsT=wz[:, 0:P], rhs=wz[:, :],
                        start=(w == 0), stop=(w == 13),
                    )
                for th in range(2):
                    t0 = th * 1024
                    # Stripe-0 weights before the x half so the first
                    # matmuls aren't queued behind the 4MB x DMA.
                    gbuf0 = wpool.tile([P, KD * P], BF16, tag="gb", name=f"gb{th}_0")
                    dbuf0 = wpool.tile([P, KD * P], BF16, tag="db", name=f"db{th}_0")
                    nc.sync.dma_start(out=gbuf0[:, :], in_=g3_ext[:, 0:KD * P])
                    nc.sync.dma_start(out=dbuf0[:, :], in_=d3_ext[:, 0:KD * P])
                    # Two half-tiles (k 0-7 / 8-15): th=1's reload of the low
                    # half only WARs against th=0's k<8 readers, which finish
                    # ~7us before the stripe ends — hides the 4MB x reload.
                    # DMA-instruction issue costs ~585ns each on the sync
                    # queue, so x is batched in 2- and 4-chunk groups and
                    # stripe 1's weights are interleaved right after the first
                    # x group to keep stripe 1 off the critical path.
                    xlo = xpool.tile([P, KD // 2, 1024], BF16, tag="xlo", name=f"xlo{th}")
                    xhi = xpool.tile([P, KD // 2, 1024], BF16, tag="xhi", name=f"xhi{th}")
                    # Critical path first: the k=0 chunk alone, so the first
                    # matmul isn't bandwidth-sharing with the bulk transfers.
                    nc.sync.dma_start(out=xlo[:, 0:1, :], in_=xt_r[:, 0:1, t0:t0 + 1024])
                    nc.sync.dma_start(out=xlo[:, 1:4, :], in_=xt_r[:, 1:4, t0:t0 + 1024])
                    if th == 0:
                        gbuf1 = wpool.tile([P, KD * P], BF16, tag="gb", name="gb0_1")
                        dbuf1 = wpool.tile([P, KD * P], BF16, tag="db", name="db0_1")
                        nc.sync.dma_start(out=gbuf1[:, :], in_=g3_ext[:, KD * P:2 * KD * P])
                        nc.sync.dma_start(out=dbuf1[:, :], in_=d3_ext[:, KD * P:2 * KD * P])
                    nc.sync.dma_start(out=xlo[:, 4:8, :], in_=xt_r[:, 4:8, t0:t0 + 1024])
                    nc.sync.dma_start(out=xhi[:, 0:4, :], in_=xt_r[:, 8:12, t0:t0 + 1024])
                    nc.sync.dma_start(out=xhi[:, 4:8, :], in_=xt_r[:, 12:16, t0:t0 + 1024])
                    if th == 0:
                        # Early load of phase-2's first up chunks (no deps).
                        nc.sync.dma_start(out=uhead[:, :, :], in_=up_r[:, 0:UH, 0:512])

                    for i in range(KH):
                        if i == 0:
                            gbuf, dbuf = gbuf0, dbuf0
                        elif th == 0 and i == 1:
                            gbuf, dbuf = gbuf1, dbuf1
                        else:
                            gbuf = wpool.tile([P, KD * P], BF16, tag="gb", name=f"gb{th}_{i}")
                            dbuf = wpool.tile([P, KD * P], BF16, tag="db", name=f"db{th}_{i}")
                            nc.sync.dma_start(out=gbuf[:, :], in_=g3_ext[:, i * KD * P:(i + 1) * KD * P])
                            nc.sync.dma_start(out=dbuf[:, :], in_=d3_ext[:, i * KD * P:(i + 1) * KD * P])

                        pg = [ps1.tile([P, 512], F32, tag="ps", name=f"pg{th}_{i}_{t2}") for t2 in range(2)]
                        pd = [ps1.tile([P, 512], F32, tag="ps", name=f"pd{th}_{i}_{t2}") for t2 in range(2)]
                        for k in range(KD):
                            st, sp = k == 0, k == KD - 1
                            xb_k = xlo[:, k, :] if k < KD // 2 else xhi[:, k - KD // 2, :]
                            for t2 in range(2):
                                nc.tensor.matmul(
                                    pg[t2][:, :], lhsT=gbuf[:, k * P:(k + 1) * P],
                                    rhs=xb_k[:, t2 * 512:(t2 + 1) * 512],
                                    start=st, stop=sp,
                                )
                            for t2 in range(2):
                                nc.tensor.matmul(
                                    pd[t2][:, :], lhsT=dbuf[:, k * P:(k + 1) * P],
                                    rhs=xb_k[:, t2 * 512:(t2 + 1) * 512],
                                    start=st, stop=sp,
                                )
                        for t2 in range(2):
                            sg = spool.tile([P, 512], F32, tag="sg", name=f"sg{th}_{i}_{t2}")
                            nc.scalar.activation(out=sg[:, :], in_=pg[t2][:, :], func=SILU, scale=1.0)
                            nc.vector.tensor_mul(
                                htbuf[:, i, t0 + t2 * 512:t0 + (t2 + 1) * 512],
                                pd[t2][:, :], sg[:, :],
                            )

            # ---- Phase 2: outT[d, t] = sum_h up[h, d] * htbuf[h, t]
            with tc.tile_pool(name="upool", bufs=2) as upool, \
                 tc.tile_pool(name="opool", bufs=3) as opool:
                ps2 = psp
                for q in range(4):
                    uq = upool.tile([P, KH, 512], BF16, tag="uq", name=f"uq{q}")
                    for kg in range(UH if q == 0 else 0, KH, 4):
                        nc.sync.dma_start(
                            out=uq[:, kg:kg + 4, :],
                            in_=up_r[:, kg:kg + 4, q * 512:(q + 1) * 512],
                        )
                    for dc in range(4):
                        po = [ps2.tile([P, 512], F32, tag="ps", name=f"po{q}_{dc}_{t4}") for t4 in range(4)]
                        for k in range(KH):
                            st, sp = k == 0, k == KH - 1
                            lh = (uhead[:, k, dc * P:(dc + 1) * P] if q == 0 and k < UH
                                  else uq[:, k, dc * P:(dc + 1) * P])
                            for t4 in range(4):
                                nc.tensor.matmul(
                                    po[t4][:, :], lhsT=lh,
                                    rhs=htbuf[:, k, t4 * 512:(t4 + 1) * 512],
                                    start=st, stop=sp,
                                )
                        drow = q * 512 + dc * P
                        for t4 in range(4):
                            oc = opool.tile([P, 512], BF16, tag="oc", name=f"oc{q}_{dc}_{t4}")
                            # Alternate Scalar/DVE for the PSUM drain so the
                            # four copies run pairwise-parallel (halves the
                            # final-column tail).
                            if t4 % 2 == 0:
                                nc.scalar.activation(out=oc[:, :], in_=po[t4][:, :], func=COPY, scale=1.0)
                            else:
                                nc.vector.tensor_scalar_add(oc[:, :], po[t4][:, :], 0.0)
                            nc.sync.dma_start(
                                out=outT_ext[drow:drow + P, t4 * 512:(t4 + 1) * 512],
                                in_=oc[:, :],
                            )

    if split_waits:
        _split_multi_waits(nc)
    return nc


_NC = None


def kernel(x, gate_proj, down_proj, up_proj, **run_kwargs):
    global _NC
    if _NC is None:
        _NC = _build()
    in_maps = []
    for e in range(E):
        xt = np.asarray(x[e], dtype=np.float32).T.astype(NPBF)
        g3 = (np.asarray(gate_proj[e], dtype=np.float32)
              .reshape(KD, P, KH, P).transpose(1, 2, 0, 3)
              .astype(NPBF).reshape(P, KH * KD * P))
        d3 = (np.asarray(down_proj[e], dtype=np.float32)
              .reshape(KD, P, KH, P).transpose(1, 2, 0, 3)
              .astype(NPBF).reshape(P, KH * KD * P))
        up = np.asarray(up_proj[e], dtype=np.float32).astype(NPBF)
        in_maps.append({
            "xt": np.ascontiguousarray(xt),
            "g3": np.ascontiguousarray(g3),
            "d3": np.ascontiguousarray(d3),
            "up": np.ascontiguousarray(up),
        })
    res = run_bass_kernel_spmd(_NC, in_maps, core_ids=list(range(E)), **run_kwargs)
    out = np.stack([
        res.results[e]["outT"].astype(np.float32).T for e in range(E)
    ])
    if run_kwargs:
        kernel.last_result = res
    return out
